# revision 1
# baseline (speedup 1.0000x reference)
"""Trainium2 Bass kernel for nn_MinimumErrorRateLoss.

Computes, for logits (B,P,H,C), ref (B,P,R), hyp (B,P,H):
    loss = mean_{b,p}[ (er - mean_p er) * softmax_p(log_probs) + 0.01 * ce ]
where
    er        = levenshtein(ref, hyp) / R
    log_probs = sum_h (logits[h, hyp[h]] - logsumexp_c logits[h, :])
    ce        = sum_{s<100} (logsumexp_c logits[s, :] - logits[s, ref[s]])

Sharding: data-parallel over the batch dim across 8 NeuronCores (4 batches
each).  Per core the kernel streams its 64 (b,p) tiles of [128,1024] logits
through SBUF once (ScalarE exp+accumulate -> logsumexp), gathers the
hyp/ref-indexed logits elements with a single indirect DMA, and runs the
edit-distance DP on VectorE with two chained instructions per hyp step
(a fused add and a tensor_tensor_scan max-recurrence), using the
transformation Y[i,j] = j + i - D[i,j] which turns the row update into
    Y_i[j] = max(Y_{i-1}[j-1] + 1 + eq[i,j], Y_i[j-1], Y_{i-1}[j])
with Y_i[0] = 0 for all i.
"""

import numpy as np

B, P, H, R, C = 32, 16, 128, 100, 1024
NCORES = 8
BL = B // NCORES  # local batches per core
NT = BL * P       # tiles (sequences) per core
NG = H + R        # gathered elements per tile (128 hyp + 100 ref)

_CACHE = {}


def _build_program():
    import concourse.bass as bass
    import concourse.bacc as bacc
    import concourse.tile as tile
    import concourse.mybir as mybir

    f32 = mybir.dt.float32
    Alu = mybir.AluOpType
    Act = mybir.ActivationFunctionType

    nc = bacc.Bacc("TRN2", target_bir_lowering=False, debug=False)

    logits_d = nc.dram_tensor("logits", [NT, H, C], f32, kind="ExternalInput")
    ref_d = nc.dram_tensor("ref_f32", [NT, R], f32, kind="ExternalInput")
    hyp_d = nc.dram_tensor("hyp_f32", [NT, H], f32, kind="ExternalInput")
    offs_d = nc.dram_tensor("offs", [NT, NG], mybir.dt.int32, kind="ExternalInput")
    mask_d = nc.dram_tensor("mask", [H, 2], f32, kind="ExternalInput")
    out_d = nc.dram_tensor("contrib", [BL, P], f32, kind="ExternalOutput")

    with tile.TileContext(nc) as tc:
        with (
            tc.tile_pool(name="persist", bufs=1) as pp,
            tc.tile_pool(name="lt", bufs=4) as ltp,
            tc.tile_pool(name="scratch", bufs=2) as scp,
            tc.tile_pool(name="psum", bufs=1, space="PSUM") as psp,
        ):
            # ---------------- DP inputs and serial chain (VectorE) ----------
            ref_sb = pp.tile([NT, R], f32)
            hyp_sb = pp.tile([NT, H], f32)
            nc.sync.dma_start(out=ref_sb[:], in_=ref_d[:])
            nc.sync.dma_start(out=hyp_sb[:], in_=hyp_d[:])

            eqm = pp.tile([NT, H, R], f32)
            ra, ha = ref_sb[:], hyp_sb[:]
            ref_bc = bass.AP(tensor=ra.tensor, offset=ra.offset,
                             ap=[ra.ap[0], [0, H], ra.ap[1]])
            hyp_bc = bass.AP(tensor=ha.tensor, offset=ha.offset,
                             ap=[ha.ap[0], ha.ap[1], [0, R]])
            nc.vector.tensor_tensor(out=eqm[:], in0=ref_bc, in1=hyp_bc,
                                    op=Alu.is_equal)

            ya = pp.tile([NT, R + 1], f32)
            yb = pp.tile([NT, R + 1], f32)
            ab = pp.tile([NT, R], f32)
            nc.vector.memset(ya[:], 0.0)
            nc.vector.memset(yb[:, 0:1], 0.0)

            bufs = [ya, yb]
            for s in range(H):
                yp = bufs[s % 2]
                yn = bufs[(s + 1) % 2]
                # A[j] = Yprev[j-1] + 1 + eq[s, j],   j = 1..R
                nc.vector.scalar_tensor_tensor(
                    out=ab[:], in0=yp[:, 0:R], scalar=1.0, in1=eqm[:, s, :],
                    op0=Alu.add, op1=Alu.add)
                # Ynew[j] = max(A[j], Ynew[j-1], Yprev[j]),  Ynew[0] = 0
                nc.vector.tensor_tensor_scan(
                    out=yn[:, 1:R + 1], data0=ab[:], data1=yp[:, 1:R + 1],
                    initial=0.0, op0=Alu.max, op1=Alu.max)

            yfin = bufs[H % 2]
            pack = pp.tile([NT, 4], f32)
            # er = dist/R = (R + H - Y)/R = 2.28 - 0.01*Y
            nc.vector.tensor_scalar(
                out=pack[:, 0:1], in0=yfin[:, R:R + 1],
                scalar1=-1.0 / R, scalar2=float(R + H) / R,
                op0=Alu.mult, op1=Alu.add)

            # ---------------- gathers (indirect DMA on gpsimd) --------------
            offs_sb = pp.tile([NT, NG], mybir.dt.int32)
            gath = pp.tile([NT, NG], f32)
            nc.sync.dma_start(out=offs_sb[:], in_=offs_d[:])
            flat = logits_d.ap().rearrange("a b c -> (a b) c")
            nc.gpsimd.indirect_dma_start(
                out=gath[:], out_offset=None, in_=flat,
                in_offset=bass.IndirectOffsetOnAxis(ap=offs_sb[:], axis=1))
            shyp = pp.tile([NT, 1], f32)
            sref = pp.tile([NT, 1], f32)
            nc.vector.reduce_sum(out=shyp[:], in_=gath[:, 0:H],
                                 axis=mybir.AxisListType.X)
            nc.vector.reduce_sum(out=sref[:], in_=gath[:, H:NG],
                                 axis=mybir.AxisListType.X)

            # ---------------- logsumexp stream (ScalarE + DMA) --------------
            sumexp = pp.tile([H, NT], f32)
            for t in range(NT):
                lt = ltp.tile([H, C], f32)
                nc.sync.dma_start(out=lt[:], in_=logits_d[t, :, :])
                sc = scp.tile([H, C], f32)
                nc.scalar.activation(out=sc[:], in_=lt[:], func=Act.Exp,
                                     accum_out=sumexp[:, t:t + 1])
            logz = pp.tile([H, NT], f32)
            nc.scalar.activation(out=logz[:], in_=sumexp[:], func=Act.Ln)

            mask_sb = pp.tile([H, 2], f32)
            nc.sync.dma_start(out=mask_sb[:], in_=mask_d[:])
            mm = psp.tile([NT, 2], f32, space="PSUM")
            nc.tensor.matmul(out=mm[:], lhsT=logz[:], rhs=mask_sb[:],
                             start=True, stop=True)
            mm_sb = pp.tile([NT, 2], f32)
            nc.vector.tensor_copy(out=mm_sb[:], in_=mm[:])

            # lp = Shyp - SlogZ_all ; ce = SlogZ_100 - Sref
            nc.vector.tensor_tensor(out=pack[:, 1:2], in0=shyp[:],
                                    in1=mm_sb[:, 0:1], op=Alu.subtract)
            nc.vector.tensor_tensor(out=pack[:, 2:3], in0=mm_sb[:, 1:2],
                                    in1=sref[:], op=Alu.subtract)
            nc.vector.memset(pack[:, 3:4], 0.0)

            # ---------------- per-batch combine ([BL, P] layout) ------------
            fin = pp.tile([BL, P * 4], f32)
            nc.sync.dma_start(out=fin[:], in_=pack[:])
            fv = fin[:].rearrange("b (p k) -> b p k", k=4)
            er_ap, lp_ap, ce_ap = fv[:, :, 0], fv[:, :, 1], fv[:, :, 2]

            mer = pp.tile([BL, 1], f32)
            nc.vector.reduce_sum(out=mer[:], in_=er_ap, axis=mybir.AxisListType.X)
            nc.vector.tensor_scalar(out=mer[:], in0=mer[:], scalar1=1.0 / P,
                                    scalar2=None, op0=Alu.mult)
            erc = pp.tile([BL, P], f32)
            nc.vector.tensor_scalar(out=erc[:], in0=er_ap, scalar1=mer[:],
                                    scalar2=None, op0=Alu.subtract)

            negmx = pp.tile([BL, 1], f32)
            nc.vector.tensor_reduce(out=negmx[:], in_=lp_ap,
                                    axis=mybir.AxisListType.X, op=Alu.max,
                                    negate=True)
            ew = pp.tile([BL, P], f32)
            se = pp.tile([BL, 1], f32)
            nc.scalar.activation(out=ew[:], in_=lp_ap, func=Act.Exp,
                                 bias=negmx[:], scale=1.0, accum_out=se[:])
            inv = pp.tile([BL, 1], f32)
            nc.vector.reciprocal(out=inv[:], in_=se[:])

            t1 = pp.tile([BL, P], f32)
            nc.vector.tensor_tensor(out=t1[:], in0=erc[:], in1=ew[:],
                                    op=Alu.mult)
            nc.vector.tensor_scalar(out=t1[:], in0=t1[:], scalar1=inv[:],
                                    scalar2=None, op0=Alu.mult)
            contrib = pp.tile([BL, P], f32)
            nc.vector.scalar_tensor_tensor(out=contrib[:], in0=ce_ap,
                                           scalar=0.01, in1=t1[:],
                                           op0=Alu.mult, op1=Alu.add)
            nc.sync.dma_start(out=out_d[:], in_=contrib[:])

    nc.compile()
    return nc


def _host_prep(logits, ref, hyp):
    """Build per-core input maps."""
    logits = np.ascontiguousarray(np.asarray(logits, dtype=np.float32))
    ref = np.asarray(ref)
    hyp = np.asarray(hyp)

    mask = np.stack([np.ones(H, np.float32),
                     (np.arange(H) < R).astype(np.float32)], axis=1)

    in_maps = []
    for k in range(NCORES):
        sl = slice(k * BL, (k + 1) * BL)
        lg = logits[sl].reshape(NT, H, C)
        rf = ref[sl].reshape(NT, R)
        hp = hyp[sl].reshape(NT, H)
        t_idx = np.arange(NT, dtype=np.int64)[:, None]
        off_hyp = (t_idx * H + np.arange(H)[None, :]) * C + hp
        off_ref = (t_idx * H + np.arange(R)[None, :]) * C + rf
        offs = np.concatenate([off_hyp, off_ref], axis=1).astype(np.int32)
        in_maps.append({
            "logits": np.ascontiguousarray(lg),
            "ref_f32": rf.astype(np.float32),
            "hyp_f32": hp.astype(np.float32),
            "offs": np.ascontiguousarray(offs),
            "mask": mask,
        })
    return in_maps


def kernel(logits, ref, hyp, _collect=None):
    from concourse import bass_utils

    if "nc" not in _CACHE:
        _CACHE["nc"] = _build_program()
    nc = _CACHE["nc"]

    in_maps = _host_prep(logits, ref, hyp)
    kw = dict(_collect) if _collect else {}
    res = bass_utils.run_bass_kernel_spmd(
        nc, in_maps, core_ids=list(range(NCORES)), **kw)
    if _collect is not None:
        _collect["res"] = res

    total = np.float64(0.0)
    for r in res.results:
        total += np.float64(r["contrib"].astype(np.float64).sum())
    return np.asarray(total / (B * P), dtype=np.float32)


# revision 3
# speedup vs baseline: 13.8872x; 13.8872x over previous
"""Trainium2 Bass kernel for nn_MinimumErrorRateLoss.

Computes, for logits (B,P,H,C), ref (B,P,R), hyp (B,P,H):
    loss = mean_{b,p}[ (er - mean_p er) * softmax_p(log_probs) + 0.01 * ce ]
where
    er        = levenshtein(ref, hyp) / R
    log_probs = sum_h (logits[h, hyp[h]] - logsumexp_c logits[h, :])
    ce        = sum_{s<100} (logsumexp_c logits[s, :] - logits[s, ref[s]])

Sharding: data-parallel over the batch dim across 8 NeuronCores (4 batches
each).  Per core the kernel streams its 64 (b,p) tiles of [128,1024] logits
through SBUF once (ScalarE exp+accumulate -> logsumexp), gathers the
hyp/ref-indexed logits elements with a single indirect DMA, and runs the
edit-distance DP on VectorE with two chained instructions per hyp step
(a fused add and a tensor_tensor_scan max-recurrence), using the
transformation Y[i,j] = j + i - D[i,j] which turns the row update into
    Y_i[j] = max(Y_{i-1}[j-1] + 1 + eq[i,j], Y_i[j-1], Y_{i-1}[j])
with Y_i[0] = 0 for all i.
"""

import numpy as np

B, P, H, R, C = 32, 16, 128, 100, 1024
NCORES = 8
BL = B // NCORES  # local batches per core
NT = BL * P       # tiles (sequences) per core
NG = H + R        # gathered elements per tile (128 hyp + 100 ref)

_CACHE = {}


def _build_program(reps=1):
    import concourse.bass as bass
    import concourse.bacc as bacc
    import concourse.tile as tile
    import concourse.mybir as mybir

    f32 = mybir.dt.float32
    Alu = mybir.AluOpType
    Act = mybir.ActivationFunctionType

    nc = bacc.Bacc("TRN2", target_bir_lowering=False, debug=False)

    logits_d = nc.dram_tensor("logits", [NT, H, C], f32, kind="ExternalInput")
    ref_d = nc.dram_tensor("ref_f32", [NT, R], f32, kind="ExternalInput")
    hyp_d = nc.dram_tensor("hyp_f32", [NT, H], f32, kind="ExternalInput")
    offs_d = nc.dram_tensor("offs", [NT, NG], mybir.dt.int32, kind="ExternalInput")
    mask_d = nc.dram_tensor("mask", [H, 2], f32, kind="ExternalInput")
    out_d = nc.dram_tensor("contrib", [BL, P], f32, kind="ExternalOutput")

    with tile.TileContext(nc) as tc:
        with (
            tc.tile_pool(name="persist", bufs=1) as pp,
            tc.tile_pool(name="lt", bufs=4) as ltp,
            tc.tile_pool(name="scratch", bufs=2) as scp,
            tc.tile_pool(name="psum", bufs=1, space="PSUM") as psp,
        ):
            for _rep in range(reps):
                _emit_body(nc, bass, tile, mybir, f32, Alu, Act,
                           logits_d, ref_d, hyp_d, offs_d, mask_d, out_d,
                           pp, ltp, scp, psp)

    nc.compile()
    return nc


def _emit_body(nc, bass, tile, mybir, f32, Alu, Act,
               logits_d, ref_d, hyp_d, offs_d, mask_d, out_d,
               pp, ltp, scp, psp):
        if True:
            # ---------------- DP inputs and serial chain (VectorE) ----------
            ref_sb = pp.tile([NT, R], f32)
            hyp_sb = pp.tile([NT, H], f32)
            nc.sync.dma_start(out=ref_sb[:], in_=ref_d[:])
            nc.sync.dma_start(out=hyp_sb[:], in_=hyp_d[:])

            eqm = pp.tile([NT, H, R], f32)
            ra, ha = ref_sb[:], hyp_sb[:]
            ref_bc = bass.AP(tensor=ra.tensor, offset=ra.offset,
                             ap=[ra.ap[0], [0, H], ra.ap[1]])
            hyp_bc = bass.AP(tensor=ha.tensor, offset=ha.offset,
                             ap=[ha.ap[0], ha.ap[1], [0, R]])
            nc.vector.tensor_tensor(out=eqm[:], in0=ref_bc, in1=hyp_bc,
                                    op=Alu.is_equal)

            ya = pp.tile([NT, R + 1], f32)
            yb = pp.tile([NT, R + 1], f32)
            ab = pp.tile([NT, R], f32)
            nc.vector.memset(ya[:], 0.0)
            nc.vector.memset(yb[:, 0:1], 0.0)

            bufs = [ya, yb]
            for s in range(H):
                yp = bufs[s % 2]
                yn = bufs[(s + 1) % 2]
                # A[j] = Yprev[j-1] + 1 + eq[s, j],   j = 1..R
                nc.vector.scalar_tensor_tensor(
                    out=ab[:], in0=yp[:, 0:R], scalar=1.0, in1=eqm[:, s, :],
                    op0=Alu.add, op1=Alu.add)
                # Ynew[j] = max(A[j], Ynew[j-1], Yprev[j]),  Ynew[0] = 0
                nc.vector.tensor_tensor_scan(
                    out=yn[:, 1:R + 1], data0=ab[:], data1=yp[:, 1:R + 1],
                    initial=0.0, op0=Alu.max, op1=Alu.max)

            yfin = bufs[H % 2]
            pack = pp.tile([NT, 4], f32)
            # er = dist/R = (R + H - Y)/R = 2.28 - 0.01*Y
            nc.vector.tensor_scalar(
                out=pack[:, 0:1], in0=yfin[:, R:R + 1],
                scalar1=-1.0 / R, scalar2=float(R + H) / R,
                op0=Alu.mult, op1=Alu.add)

            # ---------------- gathers (indirect DMA on gpsimd) --------------
            offs_sb = pp.tile([NT, NG], mybir.dt.int32)
            gath = pp.tile([NT, NG], f32)
            nc.sync.dma_start(out=offs_sb[:], in_=offs_d[:])
            flat = logits_d.ap().rearrange("a b c -> (a b) c")
            nc.gpsimd.indirect_dma_start(
                out=gath[:], out_offset=None, in_=flat,
                in_offset=bass.IndirectOffsetOnAxis(ap=offs_sb[:], axis=1))
            shyp = pp.tile([NT, 1], f32)
            sref = pp.tile([NT, 1], f32)
            nc.vector.reduce_sum(out=shyp[:], in_=gath[:, 0:H],
                                 axis=mybir.AxisListType.X)
            nc.vector.reduce_sum(out=sref[:], in_=gath[:, H:NG],
                                 axis=mybir.AxisListType.X)

            # ---------------- logsumexp stream (ScalarE + DMA) --------------
            sumexp = pp.tile([H, NT], f32)
            for t in range(NT):
                lt = ltp.tile([H, C], f32)
                nc.sync.dma_start(out=lt[:], in_=logits_d[t, :, :])
                sc = scp.tile([H, C], f32)
                nc.scalar.activation(out=sc[:], in_=lt[:], func=Act.Exp,
                                     accum_out=sumexp[:, t:t + 1])
            logz = pp.tile([H, NT], f32)
            nc.scalar.activation(out=logz[:], in_=sumexp[:], func=Act.Ln)

            mask_sb = pp.tile([H, 2], f32)
            nc.sync.dma_start(out=mask_sb[:], in_=mask_d[:])
            mm = psp.tile([NT, 2], f32, space="PSUM")
            nc.tensor.matmul(out=mm[:], lhsT=logz[:], rhs=mask_sb[:],
                             start=True, stop=True)
            mm_sb = pp.tile([NT, 2], f32)
            nc.vector.tensor_copy(out=mm_sb[:], in_=mm[:])

            # lp = Shyp - SlogZ_all ; ce = SlogZ_100 - Sref
            nc.vector.tensor_tensor(out=pack[:, 1:2], in0=shyp[:],
                                    in1=mm_sb[:, 0:1], op=Alu.subtract)
            nc.vector.tensor_tensor(out=pack[:, 2:3], in0=mm_sb[:, 1:2],
                                    in1=sref[:], op=Alu.subtract)
            nc.vector.memset(pack[:, 3:4], 0.0)

            # ---------------- per-batch combine ([BL, P] layout) ------------
            fin = pp.tile([BL, P * 4], f32)
            nc.sync.dma_start(out=fin[:], in_=pack[:])
            fv = fin[:].rearrange("b (p k) -> b p k", k=4)
            er_ap, lp_ap, ce_ap = fv[:, :, 0], fv[:, :, 1], fv[:, :, 2]

            mer = pp.tile([BL, 1], f32)
            nc.vector.reduce_sum(out=mer[:], in_=er_ap, axis=mybir.AxisListType.X)
            nc.vector.tensor_scalar(out=mer[:], in0=mer[:], scalar1=1.0 / P,
                                    scalar2=None, op0=Alu.mult)
            erc = pp.tile([BL, P], f32)
            nc.vector.tensor_scalar(out=erc[:], in0=er_ap, scalar1=mer[:],
                                    scalar2=None, op0=Alu.subtract)

            negmx = pp.tile([BL, 1], f32)
            nc.vector.tensor_reduce(out=negmx[:], in_=lp_ap,
                                    axis=mybir.AxisListType.X, op=Alu.max,
                                    negate=True)
            ew = pp.tile([BL, P], f32)
            se = pp.tile([BL, 1], f32)
            nc.scalar.activation(out=ew[:], in_=lp_ap, func=Act.Exp,
                                 bias=negmx[:], scale=1.0, accum_out=se[:])
            inv = pp.tile([BL, 1], f32)
            nc.vector.reciprocal(out=inv[:], in_=se[:])

            t1 = pp.tile([BL, P], f32)
            nc.vector.tensor_tensor(out=t1[:], in0=erc[:], in1=ew[:],
                                    op=Alu.mult)
            nc.vector.tensor_scalar(out=t1[:], in0=t1[:], scalar1=inv[:],
                                    scalar2=None, op0=Alu.mult)
            contrib = pp.tile([BL, P], f32)
            nc.vector.scalar_tensor_tensor(out=contrib[:], in0=ce_ap,
                                           scalar=0.01, in1=t1[:],
                                           op0=Alu.mult, op1=Alu.add)
            nc.sync.dma_start(out=out_d[:], in_=contrib[:])


def _host_prep(logits, ref, hyp):
    """Build per-core input maps."""
    logits = np.ascontiguousarray(np.asarray(logits, dtype=np.float32))
    ref = np.asarray(ref)
    hyp = np.asarray(hyp)

    mask = np.stack([np.ones(H, np.float32),
                     (np.arange(H) < R).astype(np.float32)], axis=1)

    in_maps = []
    for k in range(NCORES):
        sl = slice(k * BL, (k + 1) * BL)
        lg = logits[sl].reshape(NT, H, C)
        rf = ref[sl].reshape(NT, R)
        hp = hyp[sl].reshape(NT, H)
        t_idx = np.arange(NT, dtype=np.int64)[:, None]
        off_hyp = (t_idx * H + np.arange(H)[None, :]) * C + hp
        off_ref = (t_idx * H + np.arange(R)[None, :]) * C + rf
        offs = np.concatenate([off_hyp, off_ref], axis=1).astype(np.int32)
        in_maps.append({
            "logits": np.ascontiguousarray(lg),
            "ref_f32": rf.astype(np.float32),
            "hyp_f32": hp.astype(np.float32),
            "offs": np.ascontiguousarray(offs),
            "mask": mask,
        })
    return in_maps


def kernel(logits, ref, hyp, _collect=None):
    from concourse import bass_utils

    if "nc" not in _CACHE:
        _CACHE["nc"] = _build_program()
    nc = _CACHE["nc"]

    in_maps = _host_prep(logits, ref, hyp)
    kw = dict(_collect) if _collect else {}
    res = bass_utils.run_bass_kernel_spmd(
        nc, in_maps, core_ids=list(range(NCORES)), **kw)
    if _collect is not None:
        _collect["res"] = res

    total = np.float64(0.0)
    for r in res.results:
        total += np.float64(r["contrib"].astype(np.float64).sum())
    return np.asarray(total / (B * P), dtype=np.float32)
